# revision 19
# baseline (speedup 1.0000x reference)
"""Causal self-attention (B=4, T=2048, C=1024, H=16) on 8 TRN2 NeuronCores.

Sharding: tensor-parallel over heads — 2 heads per core. Each core gets the
full x (replicated, pre-transposed, bf16 on host), its 384-column slice of
W_attn (bf16), and its 128-row slice of W_proj (bf16); it produces a
full-shape [B*T, C] bf16 partial output which the host sums across cores in
fp32 (b_proj and the dropped V-bias term added on host).

Bias algebra: softmax((q+bq)(k+bk)) == softmax((q+bq)k) (the (q+bq)bk term
is constant along kt), so the K-bias is dropped. The V-bias contributes
bv @ W_proj, a token-independent vector, added on host. Only the Q-bias is
applied on device.

Per-core pipeline (activations feature-on-partition, "transposed"):
  1. x^T bf16 loaded per qt-chunk via the GpSimd SWDGE queue (separate from
     the output-store queue to avoid head-of-line blocking).
  2. qkv^T = W.T @ x^T (bf16 matmuls, fp32 PSUM); +bias for Q only; K and
     V share a 2-bank PSUM tile and are evacuated with one strided cast.
  3. Two XBAR DMA transposes per kt-chunk (off the PE) give each head's V;
     stored as [V0 | 1 | V1 | 1] so each head's [V_h | 1] is a contiguous
     65-col stationary (softmax denominator rides in the 65th column).
  4. Per qt-chunk j, kt-chunks in pairs sharing a 2-bank PSUM tile:
     S^T[kt, qt] = K_chunk @ Q^T causal-trimmed, h0/h1 issued adjacently
     with tile_position row-tiling; one exp ACTIVATE per (head, pair) over
     [128, <=1024] (no max subtraction — logits are O(1); the stale-PSUM
     gap between the two valid regions is exp'd but never read); tril mask
     multiplied into diagonal blocks post-exp (bf16, cheap); PV
     y_aug^T += [V|1].T @ P^T accumulated over kt with causal-trimmed
     moving slices.
  5. y^T = y_aug[d] * recip(y_aug[sum]); recip via DVE
     reciprocal_approx_fast, broadcast across partitions via GpSimd.
  6. out[t, :] = y^T.T @ W_proj (bf16), bf16 out, DMA'd to DRAM.
"""

import sys
import numpy as np

if "/opt/trn_rl_repo" not in sys.path:
    sys.path.insert(0, "/opt/trn_rl_repo")

from contextlib import ExitStack

import ml_dtypes
import concourse.bass as bass
import concourse.mybir as mybir
import concourse.tile as tile
from concourse import bacc
import concourse.bass_utils as _bu
from concourse.bass_utils import run_bass_kernel_spmd
from concourse.masks import make_identity

B, T, C, H, D = 4, 2048, 1024, 16, 64
P = 128
NCORES = 8
HPC = H // NCORES          # 2 heads per core
FC = HPC * D               # 128 features per core per q/k/v
NT = B * T                 # 8192 tokens
CC = C // P                # 8 contraction chunks
TJ = 512                   # token tile (free dim) for big matmuls
NQ = T // TJ               # 4 qt chunks per batch
KCH = T // P               # 16 kt chunks per batch
F32 = mybir.dt.float32
BF16 = mybir.dt.bfloat16
AF = mybir.ActivationFunctionType

_CACHE = {}


def build_program():
    nc = bacc.Bacc("TRN2", target_bir_lowering=False, debug=False)

    x_d = nc.dram_tensor("xt", [C, NT], BF16, kind="ExternalInput").ap()
    wa_d = nc.dram_tensor("w_attn", [C, 3 * FC], BF16, kind="ExternalInput").ap()
    bq_d = nc.dram_tensor("b_q", [FC, 1], F32, kind="ExternalInput").ap()
    wp_d = nc.dram_tensor("w_proj", [FC, C], BF16, kind="ExternalInput").ap()
    mk_d = nc.dram_tensor("tril", [P, P], BF16, kind="ExternalInput").ap()
    out_d = nc.dram_tensor("out", [NT, C], BF16, kind="ExternalOutput").ap()

    with tile.TileContext(nc) as tc, ExitStack() as ctx:
        consts = ctx.enter_context(tc.tile_pool(name="consts", bufs=1))
        xt_pool = ctx.enter_context(tc.tile_pool(name="xt", bufs=2))
        qkvt_pool = ctx.enter_context(tc.tile_pool(name="qkvt", bufs=2))
        vaug_pool = ctx.enter_context(tc.tile_pool(name="vaug", bufs=2))
        pt_pool = ctx.enter_context(tc.tile_pool(name="pt", bufs=4))
        sums_pool = ctx.enter_context(tc.tile_pool(name="sums", bufs=4))
        rbc_pool = ctx.enter_context(tc.tile_pool(name="rbc", bufs=2))
        y_pool = ctx.enter_context(tc.tile_pool(name="y", bufs=2))
        o_pool = ctx.enter_context(tc.tile_pool(name="o", bufs=3))

        # PSUM budget (8 banks): one uniform working pool of [128,2,512]
        # 2-bank tiles, bufs=3 (6 banks) shared by phase-B psf, scores, and
        # out-proj; plus ps_y 2x1=2 for the PV accumulators. The 3-deep
        # rotation gives the QK head-pairs enough slack to issue adjacently.
        ps_s = ctx.enter_context(tc.tile_pool(name="ps_s", bufs=3, space="PSUM"))
        ps_y = ctx.enter_context(tc.tile_pool(name="ps_y", bufs=2, space="PSUM"))

        # --- constants ---
        w_sb = consts.tile([P, CC, 3 * FC], BF16)
        nc.sync.dma_start(w_sb[:], wa_d.rearrange("(cc p) f -> p cc f", p=P))
        bq_sb = consts.tile([P, 1], F32)
        nc.sync.dma_start(bq_sb[:], bq_d)
        wp_sb = consts.tile([P, C], BF16)
        nc.sync.dma_start(wp_sb[:], wp_d)
        trilb = consts.tile([P, P], BF16)
        nc.sync.dma_start(trilb[:], mk_d)
        ident = consts.tile([P, P], F32)
        make_identity(nc, ident[:])
        identb = consts.tile([P, P], BF16)
        nc.vector.tensor_copy(out=identb[:], in_=ident[:])
        ones_b = consts.tile([P, 1], BF16)
        nc.vector.memset(ones_b[:], 1.0)

        # warm up the PE clock gate while initial DMAs land
        ps_warm = ps_s.tile([P, 2, TJ], F32, name="warm", tag="pss")
        for _ in range(40):
            nc.tensor.matmul(ps_warm[:, 0, :P], identb[:], identb[:],
                             start=True, stop=True)

        xT = x_d.rearrange("(cc p) (b j t) -> p cc b j t", p=P, b=B, j=NQ)

        for b in range(B):
            t0 = b * T

            # ---- phase A: x^T via GpSimd SWDGE queue, per qt chunk ----
            xtb = xt_pool.tile([P, CC, T], BF16)
            for j in range(NQ):
                nc.gpsimd.dma_start(
                    xtb[:, :, j * TJ:(j + 1) * TJ], xT[:, :, b, j]
                )

            # ---- phase B: qkv^T = W.T @ x^T ----
            qkvt = qkvt_pool.tile([P, 3, T], BF16)
            for tj in range(NQ):
                tsl = slice(tj * TJ, (tj + 1) * TJ)
                psq = ps_s.tile([P, 2, TJ], F32, name="psq", tag="pss")
                for cc in range(CC):
                    nc.tensor.matmul(
                        psq[:, 0, :],
                        w_sb[:, cc, 0:P],
                        xtb[:, cc, tsl],
                        start=(cc == 0),
                        stop=(cc == CC - 1),
                    )
                nc.vector.tensor_scalar_add(qkvt[:, 0, tsl], psq[:, 0, :], bq_sb[:])
                pskv = ps_s.tile([P, 2, TJ], F32, name="pskv", tag="pss")
                for f in (1, 2):
                    for cc in range(CC):
                        nc.tensor.matmul(
                            pskv[:, f - 1, :],
                            w_sb[:, cc, f * P:(f + 1) * P],
                            xtb[:, cc, tsl],
                            start=(cc == 0),
                            stop=(cc == CC - 1),
                        )
                # one strided cast evacuates K and V together
                nc.any.tensor_copy(out=qkvt[:, 1:3, tsl], in_=pskv[:])

            # ---- phase B2: V^T -> vaug = [V0 | 1 | V1 | 1] per kt-chunk ----
            vaug = vaug_pool.tile([P, KCH, 2 * (D + 1)], BF16)
            for oc in (D, 2 * D + 1):
                nc.vector.tensor_copy(
                    out=vaug[:, :, oc:oc + 1],
                    in_=ones_b[:, None, :].to_broadcast((P, KCH, 1)),
                )
            for kc in range(KCH):
                pst = ps_s.tile([P, P], BF16, name="pst", tag="pss")
                nc.tensor.transpose(
                    pst[:], qkvt[:, 2, kc * P:(kc + 1) * P], identb[:]
                )
                nc.any.tensor_copy(out=vaug[:, kc, :D], in_=pst[:, :D])
                nc.any.tensor_copy(out=vaug[:, kc, D + 1:2 * D + 1], in_=pst[:, D:])

            # ---- phase C: scores, exp, PV per qt chunk ----
            for j in range(NQ):
                nkc = 4 * j + 4

                def lo_of(kc):
                    r = kc - 4 * j
                    return r * P if r > 0 else 0

                psy = [
                    ps_y.tile([P, TJ], F32, name=f"psy{h}", tag="psy")
                    for h in range(HPC)
                ]
                for pp in range(nkc // 2):
                    kcs = (2 * pp, 2 * pp + 1)
                    lo0 = lo_of(kcs[0])
                    pss = [
                        ps_s.tile([P, 2, TJ], F32, name=f"pss{h}", tag="pss")
                        for h in range(HPC)
                    ]
                    # QK: h0/h1 adjacent per slot -> concurrent row tiles
                    for s, kc in enumerate(kcs):
                        lo = lo_of(kc)
                        for h in range(HPC):
                            hd = slice(h * D, (h + 1) * D)
                            nc.tensor.matmul(
                                pss[h][:, s, lo:],
                                qkvt[hd, 1, kc * P:(kc + 1) * P],
                                qkvt[hd, 0, j * TJ + lo:(j + 1) * TJ],
                                start=True,
                                stop=True,
                                tile_position=(h * D, 0),
                            )
                    for h in range(HPC):
                        ptp = pt_pool.tile([P, 2, TJ], BF16, name=f"pt{h}", tag="pt")
                        nc.scalar.activation(
                            ptp.rearrange("p s t -> p (s t)")[:, lo0:],
                            pss[h].rearrange("p s t -> p (s t)")[:, lo0:],
                            AF.Exp,
                            bias=0.0, scale=float(1.0 / np.sqrt(D)),
                        )
                        for s, kc in enumerate(kcs):
                            r = kc - 4 * j
                            if r >= 0:
                                blk = slice(r * P, (r + 1) * P)
                                nc.vector.tensor_mul(
                                    out=ptp[:, s, blk],
                                    in0=ptp[:, s, blk],
                                    in1=trilb[:],
                                )
                        for s, kc in enumerate(kcs):
                            lo = lo_of(kc)
                            vsl = vaug[:, kc, h * (D + 1):(h + 1) * (D + 1)]
                            nc.tensor.matmul(
                                psy[h][:D + 1, lo:],
                                vsl,
                                ptp[:, s, lo:],
                                start=(kc == 0),
                                stop=(kc == nkc - 1),
                            )

                ysb = y_pool.tile([P, TJ], BF16)
                for h in range(HPC):
                    sums = sums_pool.tile([1, TJ], F32)
                    nc.vector.tensor_copy(out=sums[:], in_=psy[h][D:D + 1, :])
                    recip = sums_pool.tile([1, TJ], F32, name="recip", tag="recip")
                    nc.vector.reciprocal_approx_fast(out=recip[:], in_=sums[:])
                    rbc = rbc_pool.tile([P, TJ], F32, tag="rbc")
                    nc.gpsimd.partition_broadcast(rbc[:D, :], recip[:])
                    nc.vector.tensor_mul(
                        out=ysb[h * D:(h + 1) * D, :],
                        in0=psy[h][:D, :],
                        in1=rbc[:D, :],
                    )

                # ---- phase D: out[t, :] = y^T.T @ W_proj ----
                for tb in range(TJ // P):
                    ost = o_pool.tile([P, C], BF16)
                    pso = ps_s.tile([P, 2, TJ], F32, name="pso", tag="pss")
                    for cn in range(C // TJ):
                        nc.tensor.matmul(
                            pso[:, cn, :],
                            ysb[:, tb * P:(tb + 1) * P],
                            wp_sb[:, cn * TJ:(cn + 1) * TJ],
                            start=True,
                            stop=True,
                        )
                    nc.any.tensor_copy(out=ost[:], in_=pso.rearrange("p s t -> p (s t)"))
                    r0 = t0 + j * TJ + tb * P
                    nc.sync.dma_start(out_d[r0:r0 + P, :], ost[:])

    nc.compile()
    return nc


def _build_tril():
    i = np.arange(P)[:, None]
    q = np.arange(P)[None, :]
    return np.ascontiguousarray((q >= i).astype(ml_dtypes.bfloat16))


def make_in_maps(x, W_attn, b_attn, W_proj):
    x_flat = np.asarray(x, dtype=np.float32).reshape(NT, C)
    x_t = np.ascontiguousarray(x_flat.T.astype(ml_dtypes.bfloat16))
    W_attn = np.asarray(W_attn, dtype=np.float32)
    b_attn = np.asarray(b_attn, dtype=np.float32)
    W_proj = np.asarray(W_proj, dtype=np.float32)
    tril = _build_tril()
    in_maps = []
    for core in range(NCORES):
        lo = core * FC
        cols = np.concatenate(
            [np.arange(lo, lo + FC) + k * C for k in range(3)]
        )
        bq = b_attn[lo:lo + FC].astype(np.float32)
        in_maps.append({
            "xt": x_t,
            "w_attn": np.ascontiguousarray(
                W_attn[:, cols].astype(ml_dtypes.bfloat16)),
            "b_q": np.ascontiguousarray(bq.reshape(FC, 1)),
            "w_proj": np.ascontiguousarray(
                W_proj[lo:lo + FC, :].astype(ml_dtypes.bfloat16)),
            "tril": tril,
        })
    return in_maps


def kernel(x, W_attn, b_attn, W_proj, b_proj, **run_kwargs):
    if "nc" not in _CACHE:
        _CACHE["nc"] = build_program()
    nc = _CACHE["nc"]
    W_attn = np.asarray(W_attn, dtype=np.float32)
    b_attn = np.asarray(b_attn, dtype=np.float32)
    W_proj = np.asarray(W_proj, dtype=np.float32)
    in_maps = make_in_maps(x, W_attn, b_attn, W_proj)
    res = run_bass_kernel_spmd(nc, in_maps, core_ids=list(range(NCORES)), **run_kwargs)
    _CACHE["last_results"] = res
    total = np.zeros((NT, C), dtype=np.float32)
    for r in res.results:
        total += np.asarray(r["out"], dtype=np.float32)
    # dropped V-bias contributes bv @ W_proj, token-independent
    bv = b_attn[2 * C:]
    total += (bv @ W_proj + np.asarray(b_proj, dtype=np.float32))[None, :]
    return total.reshape(B, T, C)


# revision 20
# speedup vs baseline: 1.3432x; 1.3432x over previous
"""Causal self-attention (B=4, T=2048, C=1024, H=16) on 8 TRN2 NeuronCores.

Sharding: tensor-parallel over heads — 2 heads per core. Each core gets the
full x (replicated, pre-transposed, bf16 on host), its 384-column slice of
W_attn (bf16), and its 128-row slice of W_proj (bf16); it produces a
full-shape [B*T, C] bf16 partial output which the host sums across cores in
fp32 (b_proj and the dropped V-bias term added on host).

Bias algebra: softmax((q+bq)(k+bk)) == softmax((q+bq)k) (the (q+bq)bk term
is constant along kt), so the K-bias is dropped. The V-bias contributes
bv @ W_proj, a token-independent vector, added on host. Only the Q-bias is
applied on device.

Per-core pipeline (activations feature-on-partition, "transposed"):
  1. x^T bf16 loaded per qt-chunk via the GpSimd SWDGE queue (separate from
     the output-store queue to avoid head-of-line blocking).
  2. qkv^T = W.T @ x^T (bf16 matmuls, fp32 PSUM); +bias for Q only.
  3. One 128x128 PE transpose per kt-chunk gives both heads' V; stored as
     [V0 | 1 | V1 | 1] so each head's [V_h | 1] is a contiguous 65-col
     stationary (softmax denominator rides in the 65th column).
  4. Per qt-chunk j, kt-chunks in pairs sharing a 2-bank PSUM tile:
     S^T[kt, qt] = K_chunk @ Q^T causal-trimmed, h0/h1 issued adjacently
     with tile_position row-tiling; one exp ACTIVATE per (head, pair) over
     [128, <=1024] (no max subtraction — logits are O(1); the stale-PSUM
     gap between the two valid regions is exp'd but never read); tril mask
     multiplied into diagonal blocks post-exp (bf16, cheap); PV
     y_aug^T += [V|1].T @ P^T accumulated over kt with causal-trimmed
     moving slices.
  5. y^T = y_aug[d] * recip(y_aug[sum]); recip via DVE
     reciprocal_approx_fast, broadcast across partitions via GpSimd.
  6. out[t, :] = y^T.T @ W_proj (bf16), bf16 out, DMA'd to DRAM.
"""

import sys
import numpy as np

if "/opt/trn_rl_repo" not in sys.path:
    sys.path.insert(0, "/opt/trn_rl_repo")

from contextlib import ExitStack

import ml_dtypes
import concourse.bass as bass
import concourse.mybir as mybir
import concourse.tile as tile
from concourse import bacc
import concourse.bass_utils as _bu
from concourse.bass_utils import run_bass_kernel_spmd
from concourse.masks import make_identity

B, T, C, H, D = 4, 2048, 1024, 16, 64
P = 128
NCORES = 8
HPC = H // NCORES          # 2 heads per core
FC = HPC * D               # 128 features per core per q/k/v
NT = B * T                 # 8192 tokens
CC = C // P                # 8 contraction chunks
TJ = 512                   # token tile (free dim) for big matmuls
NQ = T // TJ               # 4 qt chunks per batch
KCH = T // P               # 16 kt chunks per batch
F32 = mybir.dt.float32
BF16 = mybir.dt.bfloat16
AF = mybir.ActivationFunctionType

_CACHE = {}


def build_program():
    nc = bacc.Bacc("TRN2", target_bir_lowering=False, debug=False)

    x_d = nc.dram_tensor("xt", [C, NT], BF16, kind="ExternalInput").ap()
    wa_d = nc.dram_tensor("w_attn", [C, 3 * FC], BF16, kind="ExternalInput").ap()
    bq_d = nc.dram_tensor("b_q", [FC, 1], F32, kind="ExternalInput").ap()
    wp_d = nc.dram_tensor("w_proj", [FC, C], BF16, kind="ExternalInput").ap()
    mk_d = nc.dram_tensor("tril", [P, P], BF16, kind="ExternalInput").ap()
    out_d = nc.dram_tensor("out", [NT, C], BF16, kind="ExternalOutput").ap()

    with tile.TileContext(nc) as tc, ExitStack() as ctx:
        consts = ctx.enter_context(tc.tile_pool(name="consts", bufs=1))
        xt_pool = ctx.enter_context(tc.tile_pool(name="xt", bufs=2))
        qkvt_pool = ctx.enter_context(tc.tile_pool(name="qkvt", bufs=2))
        vaug_pool = ctx.enter_context(tc.tile_pool(name="vaug", bufs=2))
        pt_pool = ctx.enter_context(tc.tile_pool(name="pt", bufs=4))
        sums_pool = ctx.enter_context(tc.tile_pool(name="sums", bufs=4))
        rbc_pool = ctx.enter_context(tc.tile_pool(name="rbc", bufs=2))
        y_pool = ctx.enter_context(tc.tile_pool(name="y", bufs=2))
        o_pool = ctx.enter_context(tc.tile_pool(name="o", bufs=3))

        # PSUM budget (8 banks): ps_s 2x2=4, ps_y 2x1=2, ps_b 1, ps_o 1.
        # Separate pools for phase-B psf and phase-D pso/pst so end-of-batch
        # out-proj drain doesn't stall the next batch's QKV projection.
        ps_s = ctx.enter_context(tc.tile_pool(name="ps_s", bufs=2, space="PSUM"))
        ps_y = ctx.enter_context(tc.tile_pool(name="ps_y", bufs=2, space="PSUM"))
        ps_b = ctx.enter_context(tc.tile_pool(name="ps_b", bufs=1, space="PSUM"))
        ps_o = ctx.enter_context(tc.tile_pool(name="ps_o", bufs=1, space="PSUM"))

        # --- constants ---
        w_sb = consts.tile([P, CC, 3 * FC], BF16)
        nc.sync.dma_start(w_sb[:], wa_d.rearrange("(cc p) f -> p cc f", p=P))
        bq_sb = consts.tile([P, 1], F32)
        nc.sync.dma_start(bq_sb[:], bq_d)
        wp_sb = consts.tile([P, C], BF16)
        nc.sync.dma_start(wp_sb[:], wp_d)
        trilb = consts.tile([P, P], BF16)
        nc.sync.dma_start(trilb[:], mk_d)
        ident = consts.tile([P, P], F32)
        make_identity(nc, ident[:])
        identb = consts.tile([P, P], BF16)
        nc.vector.tensor_copy(out=identb[:], in_=ident[:])
        ones_b = consts.tile([P, 1], BF16)
        nc.vector.memset(ones_b[:], 1.0)

        # warm up the PE clock gate while initial DMAs land
        ps_warm = ps_b.tile([P, P], F32, name="warm", tag="psb")
        for _ in range(40):
            nc.tensor.matmul(ps_warm[:], identb[:], identb[:], start=True, stop=True)

        xT = x_d.rearrange("(cc p) (b j t) -> p cc b j t", p=P, b=B, j=NQ)

        for b in range(B):
            t0 = b * T

            # ---- phase A: x^T via GpSimd SWDGE queue, per qt chunk ----
            xtb = xt_pool.tile([P, CC, T], BF16)
            for j in range(NQ):
                nc.gpsimd.dma_start(
                    xtb[:, :, j * TJ:(j + 1) * TJ], xT[:, :, b, j]
                )

            # ---- phase B: qkv^T = W.T @ x^T ----
            qkvt = qkvt_pool.tile([P, 3, T], BF16)
            for tj in range(NQ):
                for f in range(3):
                    psf = ps_b.tile([P, TJ], F32, name="psf", tag="psb")
                    for cc in range(CC):
                        nc.tensor.matmul(
                            psf[:],
                            w_sb[:, cc, f * P:(f + 1) * P],
                            xtb[:, cc, tj * TJ:(tj + 1) * TJ],
                            start=(cc == 0),
                            stop=(cc == CC - 1),
                        )
                    dst = qkvt[:, f, tj * TJ:(tj + 1) * TJ]
                    if f == 0:  # Q needs its bias; K/V biases handled in math
                        nc.vector.tensor_scalar_add(dst, psf[:], bq_sb[:])
                    else:
                        nc.any.tensor_copy(out=dst, in_=psf[:])

            # ---- phase B2: V^T -> vaug = [V0 | 1 | V1 | 1] per kt-chunk ----
            vaug = vaug_pool.tile([P, KCH, 2 * (D + 1)], BF16)
            for oc in (D, 2 * D + 1):
                nc.vector.tensor_copy(
                    out=vaug[:, :, oc:oc + 1],
                    in_=ones_b[:, None, :].to_broadcast((P, KCH, 1)),
                )
            for kc in range(KCH):
                pst = ps_o.tile([P, P], BF16, name="pst", tag="pso")
                nc.tensor.transpose(
                    pst[:], qkvt[:, 2, kc * P:(kc + 1) * P], identb[:]
                )
                nc.any.tensor_copy(out=vaug[:, kc, :D], in_=pst[:, :D])
                nc.any.tensor_copy(out=vaug[:, kc, D + 1:2 * D + 1], in_=pst[:, D:])

            # ---- phase C: scores, exp, PV per qt chunk ----
            for j in range(NQ):
                nkc = 4 * j + 4

                def lo_of(kc):
                    r = kc - 4 * j
                    return r * P if r > 0 else 0

                psy = [
                    ps_y.tile([P, TJ], F32, name=f"psy{h}", tag="psy")
                    for h in range(HPC)
                ]
                for pp in range(nkc // 2):
                    kcs = (2 * pp, 2 * pp + 1)
                    lo0 = lo_of(kcs[0])
                    pss = [
                        ps_s.tile([P, 2, TJ], F32, name=f"pss{h}", tag="pss")
                        for h in range(HPC)
                    ]
                    # QK: h0/h1 adjacent per slot -> concurrent row tiles
                    for s, kc in enumerate(kcs):
                        lo = lo_of(kc)
                        for h in range(HPC):
                            hd = slice(h * D, (h + 1) * D)
                            nc.tensor.matmul(
                                pss[h][:, s, lo:],
                                qkvt[hd, 1, kc * P:(kc + 1) * P],
                                qkvt[hd, 0, j * TJ + lo:(j + 1) * TJ],
                                start=True,
                                stop=True,
                                tile_position=(h * D, 0),
                            )
                    for h in range(HPC):
                        ptp = pt_pool.tile([P, 2, TJ], BF16, name=f"pt{h}", tag="pt")
                        nc.scalar.activation(
                            ptp.rearrange("p s t -> p (s t)")[:, lo0:],
                            pss[h].rearrange("p s t -> p (s t)")[:, lo0:],
                            AF.Exp,
                            bias=0.0, scale=float(1.0 / np.sqrt(D)),
                        )
                        for s, kc in enumerate(kcs):
                            r = kc - 4 * j
                            if r >= 0:
                                blk = slice(r * P, (r + 1) * P)
                                nc.vector.tensor_mul(
                                    out=ptp[:, s, blk],
                                    in0=ptp[:, s, blk],
                                    in1=trilb[:],
                                )
                        for s, kc in enumerate(kcs):
                            lo = lo_of(kc)
                            vsl = vaug[:, kc, h * (D + 1):(h + 1) * (D + 1)]
                            nc.tensor.matmul(
                                psy[h][:D + 1, lo:],
                                vsl,
                                ptp[:, s, lo:],
                                start=(kc == 0),
                                stop=(kc == nkc - 1),
                            )

                ysb = y_pool.tile([P, TJ], BF16)
                for h in range(HPC):
                    sums = sums_pool.tile([1, TJ], F32)
                    nc.vector.tensor_copy(out=sums[:], in_=psy[h][D:D + 1, :])
                    recip = sums_pool.tile([1, TJ], F32, name="recip", tag="recip")
                    nc.vector.reciprocal_approx_fast(out=recip[:], in_=sums[:])
                    rbc = rbc_pool.tile([P, TJ], F32, tag="rbc")
                    nc.gpsimd.partition_broadcast(rbc[:D, :], recip[:])
                    nc.vector.tensor_mul(
                        out=ysb[h * D:(h + 1) * D, :],
                        in0=psy[h][:D, :],
                        in1=rbc[:D, :],
                    )

                # ---- phase D: out[t, :] = y^T.T @ W_proj ----
                for tb in range(TJ // P):
                    ost = o_pool.tile([P, C], BF16)
                    for cn in range(C // TJ):
                        pso = ps_o.tile([P, TJ], F32, name="pso", tag="pso")
                        nc.tensor.matmul(
                            pso[:],
                            ysb[:, tb * P:(tb + 1) * P],
                            wp_sb[:, cn * TJ:(cn + 1) * TJ],
                            start=True,
                            stop=True,
                        )
                        nc.any.tensor_copy(
                            out=ost[:, cn * TJ:(cn + 1) * TJ], in_=pso[:]
                        )
                    r0 = t0 + j * TJ + tb * P
                    nc.sync.dma_start(out_d[r0:r0 + P, :], ost[:])

    nc.compile()
    return nc


def _build_tril():
    i = np.arange(P)[:, None]
    q = np.arange(P)[None, :]
    return np.ascontiguousarray((q >= i).astype(ml_dtypes.bfloat16))


def make_in_maps(x, W_attn, b_attn, W_proj):
    x_flat = np.asarray(x, dtype=np.float32).reshape(NT, C)
    x_t = np.ascontiguousarray(x_flat.T.astype(ml_dtypes.bfloat16))
    W_attn = np.asarray(W_attn, dtype=np.float32)
    b_attn = np.asarray(b_attn, dtype=np.float32)
    W_proj = np.asarray(W_proj, dtype=np.float32)
    tril = _build_tril()
    in_maps = []
    for core in range(NCORES):
        lo = core * FC
        cols = np.concatenate(
            [np.arange(lo, lo + FC) + k * C for k in range(3)]
        )
        bq = b_attn[lo:lo + FC].astype(np.float32)
        in_maps.append({
            "xt": x_t,
            "w_attn": np.ascontiguousarray(
                W_attn[:, cols].astype(ml_dtypes.bfloat16)),
            "b_q": np.ascontiguousarray(bq.reshape(FC, 1)),
            "w_proj": np.ascontiguousarray(
                W_proj[lo:lo + FC, :].astype(ml_dtypes.bfloat16)),
            "tril": tril,
        })
    return in_maps


def kernel(x, W_attn, b_attn, W_proj, b_proj, **run_kwargs):
    if "nc" not in _CACHE:
        _CACHE["nc"] = build_program()
    nc = _CACHE["nc"]
    W_attn = np.asarray(W_attn, dtype=np.float32)
    b_attn = np.asarray(b_attn, dtype=np.float32)
    W_proj = np.asarray(W_proj, dtype=np.float32)
    in_maps = make_in_maps(x, W_attn, b_attn, W_proj)
    res = run_bass_kernel_spmd(nc, in_maps, core_ids=list(range(NCORES)), **run_kwargs)
    _CACHE["last_results"] = res
    total = np.zeros((NT, C), dtype=np.float32)
    for r in res.results:
        total += np.asarray(r["out"], dtype=np.float32)
    # dropped V-bias contributes bv @ W_proj, token-independent
    bv = b_attn[2 * C:]
    total += (bv @ W_proj + np.asarray(b_proj, dtype=np.float32))[None, :]
    return total.reshape(B, T, C)


# revision 23
# speedup vs baseline: 1.3562x; 1.0097x over previous
"""Causal self-attention (B=4, T=2048, C=1024, H=16) on 8 TRN2 NeuronCores.

Sharding: tensor-parallel over heads — 2 heads per core. Each core gets the
full x (replicated, pre-transposed, bf16 on host), its 384-column slice of
W_attn (bf16), and its 128-row slice of W_proj (bf16); it produces a
full-shape [B*T, C] bf16 partial output which the host sums across cores in
fp32 (b_proj and the dropped V-bias term added on host).

Bias algebra: softmax((q+bq)(k+bk)) == softmax((q+bq)k) (the (q+bq)bk term
is constant along kt), so the K-bias is dropped. The V-bias contributes
bv @ W_proj, a token-independent vector, added on host. Only the Q-bias is
applied on device.

Per-core pipeline (activations feature-on-partition, "transposed"):
  1. x^T bf16 loaded per qt-chunk via the GpSimd SWDGE queue (separate from
     the output-store queue to avoid head-of-line blocking).
  2. qkv^T = W.T @ x^T (bf16 matmuls, fp32 PSUM); +bias for Q only.
  3. One 128x128 PE transpose per kt-chunk gives both heads' V; stored as
     [V0 | 1 | V1 | 1] so each head's [V_h | 1] is a contiguous 65-col
     stationary (softmax denominator rides in the 65th column).
  4. Per qt-chunk j, kt-chunks in pairs sharing a 2-bank PSUM tile:
     S^T[kt, qt] = K_chunk @ Q^T causal-trimmed, h0/h1 issued adjacently
     with tile_position row-tiling; one exp ACTIVATE per (head, pair) over
     [128, <=1024] (no max subtraction — logits are O(1); the stale-PSUM
     gap between the two valid regions is exp'd but never read); tril mask
     multiplied into diagonal blocks post-exp (bf16, cheap); PV
     y_aug^T += [V|1].T @ P^T accumulated over kt with causal-trimmed
     moving slices.
  5. y^T = y_aug[d] * recip(y_aug[sum]); recip via DVE
     reciprocal_approx_fast, broadcast across partitions via GpSimd.
  6. out[t, :] = y^T.T @ W_proj (bf16), bf16 out, DMA'd to DRAM.
"""

import sys
import numpy as np

if "/opt/trn_rl_repo" not in sys.path:
    sys.path.insert(0, "/opt/trn_rl_repo")

from contextlib import ExitStack

import ml_dtypes
import concourse.bass as bass
import concourse.mybir as mybir
import concourse.tile as tile
from concourse import bacc
import concourse.bass_utils as _bu
from concourse.bass_utils import run_bass_kernel_spmd
from concourse.masks import make_identity

B, T, C, H, D = 4, 2048, 1024, 16, 64
P = 128
NCORES = 8
HPC = H // NCORES          # 2 heads per core
FC = HPC * D               # 128 features per core per q/k/v
NT = B * T                 # 8192 tokens
CC = C // P                # 8 contraction chunks
TJ = 512                   # token tile (free dim) for big matmuls
NQ = T // TJ               # 4 qt chunks per batch
KCH = T // P               # 16 kt chunks per batch
F32 = mybir.dt.float32
BF16 = mybir.dt.bfloat16
AF = mybir.ActivationFunctionType

_CACHE = {}


def build_program():
    nc = bacc.Bacc("TRN2", target_bir_lowering=False, debug=False)

    x_d = nc.dram_tensor("xt", [C, NT], BF16, kind="ExternalInput").ap()
    wa_d = nc.dram_tensor("w_attn", [C, 3 * FC], BF16, kind="ExternalInput").ap()
    bq_d = nc.dram_tensor("b_q", [FC, 1], F32, kind="ExternalInput").ap()
    wp_d = nc.dram_tensor("w_proj", [FC, C], BF16, kind="ExternalInput").ap()
    mk_d = nc.dram_tensor("tril", [P, P], BF16, kind="ExternalInput").ap()
    out_d = nc.dram_tensor("out", [NT, C], BF16, kind="ExternalOutput").ap()

    with tile.TileContext(nc) as tc, ExitStack() as ctx:
        consts = ctx.enter_context(tc.tile_pool(name="consts", bufs=1))
        xt_pool = ctx.enter_context(tc.tile_pool(name="xt", bufs=2))
        qkvt_pool = ctx.enter_context(tc.tile_pool(name="qkvt", bufs=2))
        vaug_pool = ctx.enter_context(tc.tile_pool(name="vaug", bufs=2))
        pt_pool = ctx.enter_context(tc.tile_pool(name="pt", bufs=6))
        sums_pool = ctx.enter_context(tc.tile_pool(name="sums", bufs=4))
        rbc_pool = ctx.enter_context(tc.tile_pool(name="rbc", bufs=2))
        y_pool = ctx.enter_context(tc.tile_pool(name="y", bufs=2))
        o_pool = ctx.enter_context(tc.tile_pool(name="o", bufs=3))

        # PSUM budget (8 banks): ps_s 2x2=4, ps_y 2x1=2, ps_b 1, ps_o 1.
        # Separate pools for phase-B psf and phase-D pso/pst so end-of-batch
        # out-proj drain doesn't stall the next batch's QKV projection.
        ps_s = ctx.enter_context(tc.tile_pool(name="ps_s", bufs=2, space="PSUM"))
        ps_y = ctx.enter_context(tc.tile_pool(name="ps_y", bufs=2, space="PSUM"))
        ps_b = ctx.enter_context(tc.tile_pool(name="ps_b", bufs=1, space="PSUM"))
        ps_o = ctx.enter_context(tc.tile_pool(name="ps_o", bufs=1, space="PSUM"))

        # --- constants ---
        w_sb = consts.tile([P, CC, 3 * FC], BF16)
        nc.sync.dma_start(w_sb[:], wa_d.rearrange("(cc p) f -> p cc f", p=P))
        bq_sb = consts.tile([P, 1], F32)
        nc.sync.dma_start(bq_sb[:], bq_d)
        wp_sb = consts.tile([P, C], BF16)
        nc.sync.dma_start(wp_sb[:], wp_d)
        trilb = consts.tile([P, P], BF16)
        nc.sync.dma_start(trilb[:], mk_d)
        ident = consts.tile([P, P], F32)
        make_identity(nc, ident[:])
        identb = consts.tile([P, P], BF16)
        nc.vector.tensor_copy(out=identb[:], in_=ident[:])
        ones_b = consts.tile([P, 1], BF16)
        nc.vector.memset(ones_b[:], 1.0)

        # warm up the PE clock gate while initial DMAs land
        ps_warm = ps_b.tile([P, P], F32, name="warm", tag="psb")
        for _ in range(40):
            nc.tensor.matmul(ps_warm[:], identb[:], identb[:], start=True, stop=True)

        xT = x_d.rearrange("(cc p) (b j t) -> p cc b j t", p=P, b=B, j=NQ)

        for b in range(B):
            t0 = b * T

            # ---- phase A: x^T via GpSimd SWDGE queue, per qt chunk ----
            xtb = xt_pool.tile([P, CC, T], BF16)
            for j in range(NQ):
                nc.gpsimd.dma_start(
                    xtb[:, :, j * TJ:(j + 1) * TJ], xT[:, :, b, j]
                )

            # ---- phase B: qkv^T = W.T @ x^T ----
            qkvt = qkvt_pool.tile([P, 3, T], BF16)
            for tj in range(NQ):
                for f in range(3):
                    psf = ps_b.tile([P, TJ], F32, name="psf", tag="psb")
                    for cc in range(CC):
                        nc.tensor.matmul(
                            psf[:],
                            w_sb[:, cc, f * P:(f + 1) * P],
                            xtb[:, cc, tj * TJ:(tj + 1) * TJ],
                            start=(cc == 0),
                            stop=(cc == CC - 1),
                        )
                    dst = qkvt[:, f, tj * TJ:(tj + 1) * TJ]
                    if f == 0:  # Q needs its bias; K/V biases handled in math
                        nc.vector.tensor_scalar_add(dst, psf[:], bq_sb[:])
                    else:
                        nc.vector.tensor_copy(out=dst, in_=psf[:])

            # ---- phase B2: V^T -> vaug = [V0 | 1 | V1 | 1] per kt-chunk ----
            vaug = vaug_pool.tile([P, KCH, 2 * (D + 1)], BF16)
            for oc in (D, 2 * D + 1):
                nc.vector.tensor_copy(
                    out=vaug[:, :, oc:oc + 1],
                    in_=ones_b[:, None, :].to_broadcast((P, KCH, 1)),
                )
            for kc in range(KCH):
                pst = ps_o.tile([P, P], BF16, name="pst", tag="pso")
                nc.tensor.transpose(
                    pst[:], qkvt[:, 2, kc * P:(kc + 1) * P], identb[:]
                )
                nc.any.tensor_copy(out=vaug[:, kc, :D], in_=pst[:, :D])
                nc.any.tensor_copy(out=vaug[:, kc, D + 1:2 * D + 1], in_=pst[:, D:])

            # ---- phase C: scores, exp, PV per qt chunk ----
            for j in range(NQ):
                nkc = 4 * j + 4

                def lo_of(kc):
                    r = kc - 4 * j
                    return r * P if r > 0 else 0

                psy = [
                    ps_y.tile([P, TJ], F32, name=f"psy{h}", tag="psy")
                    for h in range(HPC)
                ]
                for pp in range(nkc // 2):
                    kcs = (2 * pp, 2 * pp + 1)
                    lo0 = lo_of(kcs[0])
                    pss = [
                        ps_s.tile([P, 2, TJ], F32, name=f"pss{h}", tag="pss")
                        for h in range(HPC)
                    ]
                    # QK: h0/h1 adjacent per slot -> concurrent row tiles
                    for s, kc in enumerate(kcs):
                        lo = lo_of(kc)
                        for h in range(HPC):
                            hd = slice(h * D, (h + 1) * D)
                            nc.tensor.matmul(
                                pss[h][:, s, lo:],
                                qkvt[hd, 1, kc * P:(kc + 1) * P],
                                qkvt[hd, 0, j * TJ + lo:(j + 1) * TJ],
                                start=True,
                                stop=True,
                                tile_position=(h * D, 0),
                            )
                    for h in range(HPC):
                        ptp = pt_pool.tile([P, 2, TJ], BF16, name=f"pt{h}", tag="pt")
                        nc.scalar.activation(
                            ptp.rearrange("p s t -> p (s t)")[:, lo0:],
                            pss[h].rearrange("p s t -> p (s t)")[:, lo0:],
                            AF.Exp,
                            bias=0.0, scale=float(1.0 / np.sqrt(D)),
                        )
                        for s, kc in enumerate(kcs):
                            r = kc - 4 * j
                            if r >= 0:
                                blk = slice(r * P, (r + 1) * P)
                                nc.vector.tensor_mul(
                                    out=ptp[:, s, blk],
                                    in0=ptp[:, s, blk],
                                    in1=trilb[:],
                                )
                        for s, kc in enumerate(kcs):
                            lo = lo_of(kc)
                            vsl = vaug[:, kc, h * (D + 1):(h + 1) * (D + 1)]
                            nc.tensor.matmul(
                                psy[h][:D + 1, lo:],
                                vsl,
                                ptp[:, s, lo:],
                                start=(kc == 0),
                                stop=(kc == nkc - 1),
                            )

                ysb = y_pool.tile([P, TJ], BF16)
                for h in range(HPC):
                    sums = sums_pool.tile([1, TJ], F32)
                    nc.vector.tensor_copy(out=sums[:], in_=psy[h][D:D + 1, :])
                    recip = sums_pool.tile([1, TJ], F32, name="recip", tag="recip")
                    nc.vector.reciprocal_approx_fast(out=recip[:], in_=sums[:])
                    rbc = rbc_pool.tile([P, TJ], F32, tag="rbc")
                    nc.gpsimd.partition_broadcast(rbc[:D, :], recip[:])
                    nc.vector.tensor_mul(
                        out=ysb[h * D:(h + 1) * D, :],
                        in0=psy[h][:D, :],
                        in1=rbc[:D, :],
                    )

                # ---- phase D: out[t, :] = y^T.T @ W_proj ----
                for tb in range(TJ // P):
                    ost = o_pool.tile([P, C], BF16)
                    for cn in range(C // TJ):
                        pso = ps_o.tile([P, TJ], F32, name="pso", tag="pso")
                        nc.tensor.matmul(
                            pso[:],
                            ysb[:, tb * P:(tb + 1) * P],
                            wp_sb[:, cn * TJ:(cn + 1) * TJ],
                            start=True,
                            stop=True,
                        )
                        nc.vector.tensor_copy(
                            out=ost[:, cn * TJ:(cn + 1) * TJ], in_=pso[:]
                        )
                    r0 = t0 + j * TJ + tb * P
                    nc.sync.dma_start(out_d[r0:r0 + P, :], ost[:])

    nc.compile()
    return nc


def _build_tril():
    i = np.arange(P)[:, None]
    q = np.arange(P)[None, :]
    return np.ascontiguousarray((q >= i).astype(ml_dtypes.bfloat16))


def make_in_maps(x, W_attn, b_attn, W_proj):
    x_flat = np.asarray(x, dtype=np.float32).reshape(NT, C)
    x_t = np.ascontiguousarray(x_flat.T.astype(ml_dtypes.bfloat16))
    W_attn = np.asarray(W_attn, dtype=np.float32)
    b_attn = np.asarray(b_attn, dtype=np.float32)
    W_proj = np.asarray(W_proj, dtype=np.float32)
    tril = _build_tril()
    in_maps = []
    for core in range(NCORES):
        lo = core * FC
        cols = np.concatenate(
            [np.arange(lo, lo + FC) + k * C for k in range(3)]
        )
        bq = b_attn[lo:lo + FC].astype(np.float32)
        in_maps.append({
            "xt": x_t,
            "w_attn": np.ascontiguousarray(
                W_attn[:, cols].astype(ml_dtypes.bfloat16)),
            "b_q": np.ascontiguousarray(bq.reshape(FC, 1)),
            "w_proj": np.ascontiguousarray(
                W_proj[lo:lo + FC, :].astype(ml_dtypes.bfloat16)),
            "tril": tril,
        })
    return in_maps


def kernel(x, W_attn, b_attn, W_proj, b_proj, **run_kwargs):
    if "nc" not in _CACHE:
        _CACHE["nc"] = build_program()
    nc = _CACHE["nc"]
    W_attn = np.asarray(W_attn, dtype=np.float32)
    b_attn = np.asarray(b_attn, dtype=np.float32)
    W_proj = np.asarray(W_proj, dtype=np.float32)
    in_maps = make_in_maps(x, W_attn, b_attn, W_proj)
    res = run_bass_kernel_spmd(nc, in_maps, core_ids=list(range(NCORES)), **run_kwargs)
    _CACHE["last_results"] = res
    total = np.zeros((NT, C), dtype=np.float32)
    for r in res.results:
        total += np.asarray(r["out"], dtype=np.float32)
    # dropped V-bias contributes bv @ W_proj, token-independent
    bv = b_attn[2 * C:]
    total += (bv @ W_proj + np.asarray(b_proj, dtype=np.float32))[None, :]
    return total.reshape(B, T, C)
